# revision 2
# baseline (speedup 1.0000x reference)
"""GATv2 (2-layer) + linear head GNN kernel for Trainium2, 8 NeuronCores.

v2: bf16 matmuls, precomputed XL projections (stage A), one batched
indirect gather per block, node-major meta accumulation.

Per-core node space is REMAPPED: own 49 blocks first, then the other 343
blocks. srcidx1 indexes the remapped space (layer-1 gathers from the
core-private xl1_d); srcidx2 indexes the global space (layer-2 gathers
from the AllGather'd xl2full_d).
"""
import sys

sys.path.insert(0, "/opt/trn_rl_repo")

import numpy as np
import ml_dtypes
import concourse.bass as bass
import concourse.mybir as mybir
import concourse.tile as tile
from concourse import bacc
from concourse.masks import make_identity

BF16 = ml_dtypes.bfloat16

P = 128
HEADS = 4
HC = 32
H1 = HEADS * HC  # 128
C2 = 8
EDIM = 16
OUT = 8
NCORES = 8
NF = 21          # reca fields: [ex*4 | ea*16 | one]
PAD_DST = 999.0

FP = mybir.dt.float32
BF = mybir.dt.bfloat16
I32 = mybir.dt.int32


# --------------------------------------------------------------------------
# host-side preprocessing
# --------------------------------------------------------------------------

def balanced_blocks(deg, n_pad):
    import heapq

    nb = n_pad // P
    order = np.argsort(-deg, kind="stable")
    heap = [(0, b) for b in range(nb)]
    heapq.heapify(heap)
    counts = np.zeros(nb, np.int64)
    permpos = np.empty(n_pad, np.int64)
    slot_of = np.zeros(nb, np.int64)
    for node in order:
        while True:
            s, b = heapq.heappop(heap)
            if counts[b] < P:
                break
        permpos[node] = b * P + slot_of[b]
        slot_of[b] += 1
        counts[b] += 1
        if counts[b] < P:
            heapq.heappush(heap, (s + int(deg[node]), b))
    return permpos


def prep(x, edge_index, edge_attr, npc, c1c2=None, half=32768):
    n, din = x.shape
    e = edge_attr.shape[0]
    n_pad = NCORES * npc * P
    nb = n_pad // P
    HALF = min(half, n_pad)
    src = np.asarray(edge_index[0], np.int64)
    dst = np.asarray(edge_index[1], np.int64)

    deg = np.bincount(dst, minlength=n_pad).astype(np.int64)
    permpos = balanced_blocks(deg, n_pad)

    xp = np.zeros((n_pad, din), np.float32)
    xp[permpos[:n]] = np.asarray(x, np.float32)
    xpT = np.ascontiguousarray(xp.T.astype(BF16))    # [din, n_pad] global

    blk = permpos // P
    slot = permpos % P

    eb = blk[dst]
    gsrc_all = permpos[src]
    # order: by dst block, then source half (low/high), stable
    eorder = np.lexsort((gsrc_all >= HALF, eb))
    eb_s = eb[eorder]
    gsrc = gsrc_all[eorder]
    ed = dst[eorder]
    lowmask = gsrc < HALF
    n1 = np.bincount(eb_s[lowmask], minlength=nb)
    n2 = np.bincount(eb_s[~lowmask], minlength=nb)
    c1 = int(np.ceil(n1.max() / P))
    c2 = int(np.ceil(n2.max() / P))
    if c1c2 is not None:
        assert c1c2[0] >= c1 and c1c2[1] >= c2, (c1c2, c1, c2)
        c1, c2 = c1c2
    cpb = c1 + c2

    # position within block: half1 at 0.., half2 at c1*P..
    starts = np.zeros(nb + 1, np.int64)
    np.cumsum(np.bincount(eb_s, minlength=nb), out=starts[1:])
    pos_in_blk = np.arange(e) - starts[eb_s]         # order: half1 then half2
    pos = np.where(lowmask, pos_in_blk, c1 * P + pos_in_blk - n1[eb_s])
    cc = pos // P
    pp = pos % P

    g_srcl = np.zeros((nb, P, cpb), np.int64)        # half-local source idx
    g_dstf = np.full((nb, P, cpb), PAD_DST, np.float32)
    g_ea = np.zeros((nb, P, cpb, EDIM), np.float32)
    g_one = np.zeros((nb, P, cpb), np.float32)
    g_src = np.zeros((nb, P, cpb), np.int64)         # global (emulator)
    g_srcl[eb_s, pp, cc] = np.where(lowmask, gsrc, gsrc - HALF)
    g_src[eb_s, pp, cc] = gsrc
    g_dstf[eb_s, pp, cc] = slot[ed].astype(np.float32)
    g_ea[eb_s, pp, cc] = np.asarray(edge_attr, np.float32)[eorder]
    g_one[eb_s, pp, cc] = 1.0

    rd = 1.0 / np.maximum(deg.astype(np.float32), 1.0)
    rdp = np.empty(n_pad, np.float32)
    rdp[permpos] = rd
    rdt_g = rdp.reshape(nb, P).transpose(1, 0)       # [P, nb]

    def wrap16(idx_flat):
        """idx j at partition j%16, col j//16, replicated x8 -> [128, len/16]"""
        w = idx_flat.reshape(-1, 16).T               # [16, len/16]
        return np.tile(w, (8, 1))

    cores = []
    for c in range(NCORES):
        own = np.arange(c * npc, (c + 1) * npc)
        idxg = np.zeros((P, npc * cpb * 8), np.int16)
        for jj, b in enumerate(own):
            flat = g_srcl[b].T.reshape(-1)           # [cpb*P] in (cc, pp) order
            col = jj * cpb * 8
            idxg[:, col:col + c1 * 8] = wrap16(
                flat[0:c1 * P].astype(np.int16))
            idxg[:, col + c1 * 8:col + cpb * 8] = wrap16(
                flat[c1 * P:cpb * P].astype(np.int16))
        dstf = np.ascontiguousarray(
            g_dstf[own].transpose(1, 0, 2).reshape(P, npc * cpb))
        srcidx_emu = np.ascontiguousarray(
            g_src[own].transpose(1, 0, 2).reshape(P, npc * cpb))

        reca = np.zeros((npc, P, cpb, NF), np.float32)
        reca[:, :, :, 4:4 + EDIM] = g_ea[own]
        reca[:, :, :, 20] = g_one[own]
        reca = reca.reshape(npc, P, cpb * NF).astype(BF16)

        ea1 = np.concatenate([g_ea[own], g_one[own][..., None]], axis=3)
        eat = np.ascontiguousarray(
            ea1.transpose(0, 3, 2, 1)).reshape(npc, EDIM + 1, cpb * P)
        eat = eat.astype(BF16)

        rdt = np.ascontiguousarray(rdt_g[:, own])
        xpTown = np.ascontiguousarray(xpT[:, c * npc * P:(c + 1) * npc * P])
        cores.append(dict(xpT=xpT, xpTown=xpTown, idxg=idxg, dstf=dstf,
                          reca=reca, eat=eat, rdt=rdt,
                          srcidx_emu=srcidx_emu))

    return dict(cores=cores, permpos=permpos, n_pad=n_pad, nb=nb,
                cpb=cpb, c1=c1, c2=c2)


def prep_weights(w):
    f32 = lambda a: np.asarray(a, np.float32)
    bf = lambda a: np.asarray(a, np.float32).astype(BF16)
    We1a = np.concatenate(
        [f32(w["We1"]), (f32(w["b1l"]) + f32(w["b1r"]))[None, :]], axis=0)
    We2a = np.concatenate(
        [f32(w["We2"]), (f32(w["b2l"]) + f32(w["b2r"]))[None, :]], axis=0)
    attB = np.tile(f32(w["att1"]).reshape(1, -1), (P, 1))
    att2r = np.tile(f32(w["att2"]).reshape(1, -1), (P, 1))
    E4f = np.zeros((HEADS, H1), np.float32)
    for h in range(HEADS):
        E4f[h, h * HC:(h + 1) * HC] = 1.0
    return dict(
        W1l=bf(w["W1l"]), W1r=bf(w["W1r"]), We1a=bf(We1a), attB=bf(attB),
        E4f=E4f, bias1e=(f32(w["bias1"]) + f32(w["b1l"]))[:, None],
        W2l=bf(w["W2l"]), W2r=bf(w["W2r"]), We2a=bf(We2a), att2r=bf(att2r),
        bias2row=np.tile((f32(w["bias2"]) + f32(w["b2l"]))[None, :], (P, 1)),
        Wlin=bf(w["Wlin"]), blin_row=bf(f32(w["blin"])[None, :]),
    )


# --------------------------------------------------------------------------
# numpy emulator (fp32, mirrors device dataflow; debug only)
# --------------------------------------------------------------------------

def np_forward(pp, wp, npc, lrelu_on=True):
    nb, cpb = pp["nb"], pp["cpb"]
    n_pad = pp["n_pad"]

    def lrelu(v):
        return np.where(v > 0, v, 0.2 * v) if lrelu_on else v

    def elu(v):
        return np.where(v > 0, v, np.exp(np.minimum(v, 0)) - 1.0)

    f32 = lambda a: np.asarray(a, np.float32)
    W1l, W1r = f32(wp["W1l"]), f32(wp["W1r"])
    We1a, attB = f32(wp["We1a"]), f32(wp["attB"])
    W2l, W2r, We2a = f32(wp["W2l"]), f32(wp["W2r"]), f32(wp["We2a"])
    att2r, Wlin = f32(wp["att2r"]), f32(wp["Wlin"])
    blin = f32(wp["blin_row"])[0]
    att1 = attB[0]
    att2 = att2r[0]

    xpTg = f32(pp["cores"][0]["xpT"])
    XLg = xpTg.T @ W1l                               # [n_pad, 128] global
    xl2loc = np.zeros((NCORES, npc * P, C2), np.float32)
    xr2_all = np.zeros((NCORES, npc * P, C2), np.float32)
    easumT_all = np.zeros((NCORES, EDIM + 1, npc * P), np.float32)
    for c in range(NCORES):
        cd = pp["cores"][c]
        XRo = f32(cd["xpTown"]).T @ W1r
        for j in range(npc):
            sl = slice(j * cpb, (j + 1) * cpb)
            idx = cd["srcidx_emu"][:, sl]            # [P,cpb] global
            xlg = XLg[idx]
            dstf = cd["dstf"][:, sl]
            oh = (dstf[:, :, None] == np.arange(P)[None, None, :])
            oh = oh.astype(np.float32)
            XRj = XRo[j * P:(j + 1) * P]
            eaj = f32(cd["eat"][j]).reshape(EDIM + 1, cpb, P)
            ea_e = eaj.transpose(2, 1, 0)
            m = lrelu(xlg + np.einsum("pcn,nf->pcf", oh, XRj)
                      + ea_e @ We1a)
            logits = (m * att1[None, None, :]).reshape(P, cpb, HEADS, HC).sum(-1)
            ex = np.exp(logits)
            reca = f32(cd["reca"][j]).reshape(P, cpb, NF)
            rec21 = np.concatenate([ex, reca[:, :, 4:]], axis=2)
            meta = np.einsum("pcn,pcf->nf", oh, rec21)
            easum = meta[:, 4:]
            easumT_all[c][:, j * P:(j + 1) * P] = easum.T
            rdt = cd["rdt"][:, j]
            efd = (easum @ We1a) * rdt[:, None]
            XLj = XLg[(c * npc + j) * P:(c * npc + j + 1) * P]
            mloop = lrelu(XLj + XRj + efd)
            lgl = (mloop * att1[None, :]).reshape(P, HEADS, HC).sum(-1)
            exl = np.exp(lgl)
            denom = meta[:, 0:4] + exl
            xlw = xlg * np.repeat(ex, HC, axis=2)
            numerT = np.einsum("pcf,pcn->fn", xlw, oh)
            numerT = numerT + (XLj * np.repeat(exl, HC, 1)).T
            rfull = np.repeat((1.0 / denom).T, HC, axis=0)
            hT = numerT * rfull + wp["bias1e"]
            hT = elu(hT)
            h = hT.T
            xl2loc[c, j * P:(j + 1) * P] = h @ W2l
            xr2_all[c, j * P:(j + 1) * P] = h @ W2r
    xl2full = xl2loc.reshape(-1, C2)

    y = np.zeros((n_pad, OUT), np.float32)
    for c in range(NCORES):
        cd = pp["cores"][c]
        for j in range(npc):
            sl = slice(j * cpb, (j + 1) * cpb)
            idx2 = cd["srcidx_emu"][:, sl]
            xl2g = xl2full[idx2]
            dstf = cd["dstf"][:, sl]
            oh = (dstf[:, :, None] == np.arange(P)[None, None, :])
            oh = oh.astype(np.float32)
            xr2j = xr2_all[c, j * P:(j + 1) * P]
            eaj = f32(cd["eat"][j]).reshape(EDIM + 1, cpb, P)
            ea_e = eaj.transpose(2, 1, 0)
            m2 = lrelu(xl2g + np.einsum("pcn,nf->pcf", oh, xr2j)
                       + ea_e @ We2a)
            lg2 = (m2 * att2[None, None, :]).sum(-1)
            ex2 = np.exp(lg2)
            x9 = np.concatenate([xl2g * ex2[:, :, None], ex2[:, :, None]],
                                axis=2)
            meta2 = np.einsum("pcn,pcf->nf", oh, x9)
            rdt = cd["rdt"][:, j]
            ef2 = (easumT_all[c][:, j * P:(j + 1) * P].T @ We2a) * rdt[:, None]
            xl2j = xl2loc[c, j * P:(j + 1) * P]
            m2l = lrelu(xl2j + xr2j + ef2)
            ex2l = np.exp((m2l * att2[None, :]).sum(-1))
            numer2 = meta2[:, 0:8] + xl2j * ex2l[:, None]
            denom2 = meta2[:, 8] + ex2l
            o2 = numer2 / denom2[:, None] + wp["bias2row"]
            o2 = elu(o2)
            ylin = o2 @ Wlin + blin[None, :]
            y[(c * npc + j) * P:(c * npc + j + 1) * P] = \
                1.0 / (1.0 + np.exp(-ylin))
    return y


# --------------------------------------------------------------------------
# device program
# --------------------------------------------------------------------------


GPIECE = 8  # max chunks (1024 idxs) per dma_gather call


def gather_block(nc, out_t, table_d, idxg, j, c1, c2, cpb, HALF, n_pad):
    """Gather one block's source rows in <=GPIECE-chunk dma_gather calls."""
    ioff = j * cpb * 8
    for base, nchunks, lo, hi in ((0, c1, 0, HALF), (c1, c2, HALF, n_pad)):
        for s in range(0, nchunks, GPIECE):
            k = min(GPIECE, nchunks - s)
            nc.gpsimd.dma_gather(
                out_ap=out_t[:, base + s:base + s + k, :],
                in_ap=table_d[lo:hi, :],
                idxs_ap=idxg[:, ioff + (base + s) * 8:
                             ioff + (base + s + k) * 8],
                num_idxs=k * P, num_idxs_reg=k * P, elem_size=H1)


def build_nc(npc, c1, c2, n_pad, sim_safe=False, debug=False,
             half=32768):
    from concourse import library_config
    nc = bacc.Bacc("TRN2", target_bir_lowering=False)
    cpb = c1 + c2
    npcP = npc * P
    nb = n_pad // P
    HALF = min(half, n_pad)
    GB = 7                      # blocks per stage-A group
    assert npc % GB == 0 and nb % GB == 0

    xpT_d = nc.dram_tensor("xpT", [H1, n_pad], BF, kind="ExternalInput")
    xpTown_d = nc.dram_tensor("xpTown", [H1, npcP], BF, kind="ExternalInput")
    idxg_d = nc.dram_tensor("idxg", [P, npc * cpb * 8], mybir.dt.int16,
                            kind="ExternalInput")
    dstf_d = nc.dram_tensor("dstf", [P, npc * cpb], FP, kind="ExternalInput")
    reca_d = nc.dram_tensor("reca", [npc, P, cpb * NF], BF, kind="ExternalInput")
    eat_d = nc.dram_tensor("eat", [npc, EDIM + 1, cpb * P], BF,
                           kind="ExternalInput")
    rdt_d = nc.dram_tensor("rdt", [P, npc], FP, kind="ExternalInput")
    wnames = dict(
        W1l=([H1, H1], BF), W1r=([H1, H1], BF), We1a=([EDIM + 1, H1], BF),
        attB=([P, H1], BF), E4f=([HEADS, H1], FP), bias1e=([H1, 1], FP),
        W2l=([H1, C2], BF), W2r=([H1, C2], BF), We2a=([EDIM + 1, C2], BF),
        att2r=([P, C2], BF), bias2row=([P, C2], FP), Wlin=([C2, C2], BF),
        blin_row=([1, C2], BF),
    )
    wd = {k: nc.dram_tensor(k, sh, dt, kind="ExternalInput")
          for k, (sh, dt) in wnames.items()}
    y_d = nc.dram_tensor("y", [npcP, OUT], FP, kind="ExternalOutput")
    xl1_d = nc.dram_tensor("xl1", [n_pad, H1], BF,
                           kind="ExternalOutput" if debug else "Internal")
    xl2loc_d = nc.dram_tensor("xl2loc", [npcP, C2], BF)
    if debug:
        xl2dbg_d = nc.dram_tensor("xl2dbg", [npcP, C2], BF,
                                  kind="ExternalOutput")
        xlgdbg_d = nc.dram_tensor("xlgdbg", [P, cpb * H1], BF,
                                  kind="ExternalOutput")
        mdbg_d = nc.dram_tensor("mdbg", [P, cpb * H1], BF,
                                kind="ExternalOutput")
        ohdbg_d = nc.dram_tensor("ohdbg", [P, cpb * P], BF,
                                 kind="ExternalOutput")
        recdbg_d = nc.dram_tensor("recdbg", [P, cpb * NF], BF,
                                  kind="ExternalOutput")
    xl2full_d = nc.dram_tensor("xl2full", [n_pad, C2], BF, addr_space="Shared")
    xl2pad_d = nc.dram_tensor("xl2pad", [n_pad, H1], BF)

    PRELU = mybir.ActivationFunctionType.Prelu
    if sim_safe:
        PRELU = mybir.ActivationFunctionType.Copy
    EXP = mybir.ActivationFunctionType.Exp
    RELU = mybir.ActivationFunctionType.Relu
    COPY = mybir.ActivationFunctionType.Copy
    SIGM = mybir.ActivationFunctionType.Sigmoid
    ADD = mybir.AluOpType.add
    MULT = mybir.AluOpType.mult
    MIN = mybir.AluOpType.min
    ISEQ = mybir.AluOpType.is_equal

    from contextlib import ExitStack

    with tile.TileContext(nc) as tc, ExitStack() as stack, \
            nc.allow_low_precision(reason="bf16 logits reduce"):
        cp = stack.enter_context(tc.tile_pool(name="consts", bufs=1))
        bp = stack.enter_context(tc.tile_pool(name="big", bufs=2))
        gp = stack.enter_context(tc.tile_pool(name="gath", bufs=3))
        sp = stack.enter_context(tc.tile_pool(name="small", bufs=3))
        pf = stack.enter_context(tc.tile_pool(name="pf", bufs=2, space="PSUM"))
        pb = stack.enter_context(tc.tile_pool(name="pb", bufs=2, space="PSUM"))
        pn = stack.enter_context(tc.tile_pool(name="pn", bufs=1, space="PSUM"))
        pm = stack.enter_context(tc.tile_pool(name="pm", bufs=2, space="PSUM"))

        identb = cp.tile([P, P], BF)
        make_identity(nc, identb[:])
        identf = cp.tile([P, P], FP)
        make_identity(nc, identf[:])
        iota_i = cp.tile([P, P], I32)
        nc.gpsimd.iota(iota_i[:], pattern=[[1, P]], base=0, channel_multiplier=0)
        iota_b = cp.tile([P, P], BF)
        nc.vector.tensor_copy(iota_b[:], iota_i[:])
        alpha02 = cp.tile([P, 1], FP)
        nc.vector.memset(alpha02[:], 0.2 if not sim_safe else 1.0)
        ones1 = cp.tile([1, P], BF)
        nc.vector.memset(ones1[:], 1.0)
        w = {}
        for k, (sh, dt) in wnames.items():
            w[k] = cp.tile(sh, dt, name=f"w_{k}", tag=f"w_{k}")
            nc.sync.dma_start(w[k][:], wd[k][:])
        rdt = cp.tile([P, npc], FP)
        nc.sync.dma_start(rdt[:], rdt_d[:])
        idxg = cp.tile([P, npc * cpb * 8], mybir.dt.int16)
        nc.sync.dma_start(idxg[:], idxg_d[:])
        dstf = cp.tile([P, npc * cpb], FP)
        nc.sync.dma_start(dstf[:], dstf_d[:])
        nc.gpsimd.load_library(library_config.mlp)

        XLown = cp.tile([P, npc * P], BF)
        XRown = cp.tile([P, npc * P], BF)
        easumT = cp.tile([EDIM + 1, npc * P], BF)
        xl2own = cp.tile([P, npc * C2], BF)
        xr2own = cp.tile([P, npc * C2], BF)
        y_all = cp.tile([P, npc * OUT], FP)

        # ---------------- stage A: XL = x @ W1l for all blocks ------------
        ngrp_own = npc // GB
        for g in range(ngrp_own):
            xt = bp.tile([P, GB * P], BF, tag="xpT_in")
            nc.sync.dma_start(xt[:], xpTown_d[:, g * GB * P:(g + 1) * GB * P])
            for k in range(GB):
                b0 = g * GB + k
                xl_ps = pf.tile([P, 512], FP, tag="pf")
                nc.tensor.matmul(xl_ps[:, 0:H1], lhsT=xt[:, k * P:(k + 1) * P],
                                 rhs=w["W1l"][:], start=True, stop=True)
                nc.scalar.activation(XLown[:, b0 * P:(b0 + 1) * P],
                                     xl_ps[:, 0:H1], COPY)
                xr_ps = pf.tile([P, 512], FP, tag="pf")
                nc.tensor.matmul(xr_ps[:, 0:H1], lhsT=xt[:, k * P:(k + 1) * P],
                                 rhs=w["W1r"][:], start=True, stop=True)
                nc.scalar.activation(XRown[:, b0 * P:(b0 + 1) * P],
                                     xr_ps[:, 0:H1], COPY)

        ngrp = nb // GB
        for g in range(ngrp):
            off = g * GB * P
            xt = bp.tile([P, GB * P], BF, tag="xpT_in")
            nc.sync.dma_start(xt[:], xpT_d[:, off:off + GB * P])
            stg = bp.tile([P, GB * P], BF, tag="stage")
            for k in range(GB):
                xl_ps = pf.tile([P, 512], FP, tag="pf")
                nc.tensor.matmul(xl_ps[:, 0:H1], lhsT=xt[:, k * P:(k + 1) * P],
                                 rhs=w["W1l"][:], start=True, stop=True)
                nc.scalar.activation(stg[:, k * P:(k + 1) * P],
                                     xl_ps[:, 0:H1], COPY)
            nc.sync.dma_start(
                xl1_d[off:off + GB * P, :].rearrange("(b p) f -> p b f", p=P),
                stg[:].rearrange("p (b f) -> p b f", f=H1))

        # ---------------- layer 1 ----------------
        for j in range(npc):
            xlg = gp.tile([P, cpb, H1], BF, tag="xlg")
            gather_block(nc, xlg, xl1_d, idxg, j, c1, c2, cpb, HALF, n_pad)
            reca = bp.tile([P, cpb * NF], BF, tag="reca")
            nc.sync.dma_start(reca[:], reca_d[j, :, :])
            rec_v = reca[:].rearrange("p (c f) -> p c f", f=NF)
            eat = bp.tile([EDIM + 1, cpb * P], BF, tag="eat")
            nc.sync.dma_start(eat[:], eat_d[j, :, :])

            oh_all = bp.tile([P, cpb, P], BF, tag="oh_all")
            ohT_all = bp.tile([P, cpb, P], BF, tag="ohT_all")
            m_all = bp.tile([P, cpb * H1], BF, tag="m_all")
            m_v = m_all[:].rearrange("p (c f) -> p c f", f=H1)
            xlg_v = xlg[:]

            for c in range(cpb):
                nc.vector.tensor_scalar(
                    out=oh_all[:, c, :], in0=iota_b[:],
                    scalar1=dstf[:, j * cpb + c:j * cpb + c + 1],
                    scalar2=None, op0=ISEQ)
                ohT_ps = pb.tile([P, 1024], BF, tag="tpb")
                nc.tensor.transpose(out=ohT_ps[:, 0:P], in_=oh_all[:, c, :],
                                    identity=identb[:])
                nc.scalar.activation(ohT_all[:, c, :], ohT_ps[:, 0:P], COPY)
                m_ps = pm.tile([P, 512], FP, tag="m")
                nc.tensor.matmul(m_ps[:, 0:H1], lhsT=identb[:],
                                 rhs=xlg_v[:, c, :], start=True, stop=False)
                nc.tensor.matmul(m_ps[:, 0:H1], lhsT=ohT_all[:, c, :],
                                 rhs=XRown[:, j * P:(j + 1) * P],
                                 start=False, stop=False)
                nc.tensor.matmul(m_ps[:, 0:H1], lhsT=eat[:, c * P:(c + 1) * P],
                                 rhs=w["We1a"][:],
                                 start=False, stop=True)
                nc.scalar.activation(m_v[:, c, :], m_ps[:, 0:H1], PRELU,
                                     alpha=alpha02[:])

            # logits -> ex (into reca cols 0:4)
            nc.vector.tensor_tensor(
                out=m_v[:], in0=m_v[:],
                in1=w["attB"][:].unsqueeze(1).to_broadcast([P, cpb, H1]),
                op=MULT)
            logits = sp.tile([P, cpb * HEADS], BF, tag="logits")
            nc.vector.tensor_reduce(
                out=logits[:].rearrange("p (c h) -> p c h", h=HEADS),
                in_=m_all[:].rearrange("p (c h k) -> p c h k", h=HEADS, k=HC),
                axis=mybir.AxisListType.X, op=ADD)
            nc.scalar.activation(
                rec_v[:, :, 0:HEADS],
                logits[:].rearrange("p (c h) -> p c h", h=HEADS), EXP)
            # xlw = xlg * ex (in place)
            nc.vector.tensor_tensor(
                out=xlg_v.rearrange("p c (h k) -> p c h k", h=HEADS),
                in0=xlg_v.rearrange("p c (h k) -> p c h k", h=HEADS),
                in1=rec_v[:, :, 0:HEADS].unsqueeze(3)
                    .to_broadcast([P, cpb, HEADS, HC]),
                op=MULT)

            if debug and j == 0:
                nc.sync.dma_start(xlgdbg_d[:],
                                  xlg[:].rearrange("p c f -> p (c f)"))
                nc.sync.dma_start(mdbg_d[:], m_all[:])
                nc.sync.dma_start(ohdbg_d[:],
                                  oh_all[:].rearrange("p c f -> p (c f)"))
                nc.sync.dma_start(recdbg_d[:], reca[:])
            numerT_ps = pn.tile([P, 512], FP, tag="numerT")
            meta_ps = pn.tile([P, 512], FP, tag="meta")
            for c in range(cpb):
                nc.tensor.matmul(numerT_ps[:, 0:P], lhsT=xlg_v[:, c, :],
                                 rhs=oh_all[:, c, :],
                                 start=(c == 0), stop=(c == cpb - 1))
                nc.tensor.matmul(meta_ps[:, 0:NF], lhsT=oh_all[:, c, :],
                                 rhs=rec_v[:, c, :],
                                 start=(c == 0), stop=(c == cpb - 1))

            # loop chunk (self loop), node-major
            easum_nm = sp.tile([P, EDIM + 1], BF, tag="easum_nm")
            nc.scalar.activation(easum_nm[:], meta_ps[:, 4:NF], COPY)
            eaT_ps = pb.tile([P, 1024], BF, tag="tpb")
            nc.tensor.transpose(out=eaT_ps[0:EDIM + 1, 0:P], in_=easum_nm[:],
                                identity=identb[:])
            nc.scalar.activation(easumT[:, j * P:(j + 1) * P],
                                 eaT_ps[0:EDIM + 1, 0:P], COPY)
            efd_ps = pf.tile([P, 512], FP, tag="pf")
            nc.tensor.matmul(efd_ps[:, 0:H1],
                             lhsT=easumT[:, j * P:(j + 1) * P],
                             rhs=w["We1a"][:], start=True, stop=True)
            efd = sp.tile([P, H1], BF, tag="efd")
            nc.vector.tensor_scalar(out=efd[:], in0=efd_ps[:, 0:H1],
                                    scalar1=rdt[:, j:j + 1], scalar2=None,
                                    op0=MULT)
            ml_ps = pf.tile([P, 512], FP, tag="pf")
            nc.tensor.matmul(ml_ps[:, 0:H1], lhsT=identb[:],
                             rhs=XLown[:, j * P:(j + 1) * P],
                             start=True, stop=False)
            nc.tensor.matmul(ml_ps[:, 0:H1], lhsT=identb[:],
                             rhs=XRown[:, j * P:(j + 1) * P],
                             start=False, stop=False)
            nc.tensor.matmul(ml_ps[:, 0:H1], lhsT=identb[:], rhs=efd[:],
                             start=False, stop=True)
            ml = sp.tile([P, H1], BF, tag="ml")
            nc.scalar.activation(ml[:], ml_ps[:, 0:H1], PRELU, alpha=alpha02[:])
            nc.vector.tensor_tensor(out=ml[:], in0=ml[:], in1=w["attB"][:],
                                    op=MULT)
            lgl = sp.tile([P, HEADS], BF, tag="lgl")
            nc.vector.tensor_reduce(
                out=lgl[:], in_=ml[:].rearrange("p (h k) -> p h k", h=HEADS),
                axis=mybir.AxisListType.X, op=ADD)
            exl = sp.tile([P, HEADS], BF, tag="exl")
            nc.scalar.activation(exl[:], lgl[:], EXP)
            nc.tensor.matmul(meta_ps[:, 0:HEADS], lhsT=identb[:], rhs=exl[:],
                             start=False, stop=True, skip_group_check=True)
            xlwl = sp.tile([P, H1], BF, tag="xlwl")
            nc.vector.tensor_tensor(
                out=xlwl[:].rearrange("p (h k) -> p h k", h=HEADS),
                in0=XLown[:, j * P:(j + 1) * P]
                    .rearrange("p (h k) -> p h k", h=HEADS),
                in1=exl[:].unsqueeze(2).to_broadcast([P, HEADS, HC]),
                op=MULT)
            nc.tensor.matmul(numerT_ps[:, 0:P], lhsT=xlwl[:], rhs=identb[:],
                             start=False, stop=True, skip_group_check=True)

            # finalize block: hT = elu(numerT * rfull + bias1e)
            recip = sp.tile([P, HEADS], FP, tag="recip")
            nc.vector.reciprocal(recip[:], meta_ps[:, 0:HEADS])
            rcT_ps = pf.tile([P, 512], FP, tag="pf")
            nc.tensor.transpose(out=rcT_ps[0:HEADS, 0:P], in_=recip[:],
                                identity=identf[:])
            rcT = sp.tile([HEADS, P], FP, tag="rcT")
            nc.scalar.activation(rcT[:], rcT_ps[0:HEADS, 0:P], COPY)
            rfull_ps = pf.tile([P, 512], FP, tag="pf")
            nc.tensor.matmul(rfull_ps[:, 0:P], lhsT=w["E4f"][:], rhs=rcT[:],
                             start=True, stop=True)
            rfull = sp.tile([P, P], FP, tag="rfull")
            nc.scalar.activation(rfull[:], rfull_ps[:, 0:P], COPY)
            hT = sp.tile([P, P], FP, tag="hT")
            nc.vector.tensor_tensor(out=hT[:], in0=numerT_ps[:, 0:P],
                                    in1=rfull[:], op=MULT)
            tmin = sp.tile([P, P], FP, tag="tmin")
            nc.vector.tensor_scalar(out=tmin[:], in0=hT[:],
                                    scalar1=w["bias1e"][:], scalar2=0.0,
                                    op0=ADD, op1=MIN)
            ue = sp.tile([P, P], BF, tag="ue")
            nc.scalar.activation(ue[:], tmin[:], EXP)
            hTb = sp.tile([P, P], BF, tag="hTb")
            nc.scalar.activation(hTb[:], hT[:], RELU, bias=w["bias1e"][:])
            nc.vector.tensor_tensor(out=hTb[:], in0=hTb[:], in1=ue[:], op=ADD)
            nc.vector.tensor_scalar(out=hTb[:], in0=hTb[:], scalar1=-1.0,
                                    scalar2=None, op0=ADD)

            xl2_ps = pf.tile([P, 512], FP, tag="pf")
            nc.tensor.matmul(xl2_ps[:, 0:C2], lhsT=hTb[:], rhs=w["W2l"][:],
                             start=True, stop=True)
            nc.scalar.activation(xl2own[:, j * C2:(j + 1) * C2],
                                 xl2_ps[:, 0:C2], COPY)
            xr2_ps = pf.tile([P, 512], FP, tag="pf")
            nc.tensor.matmul(xr2_ps[:, 0:C2], lhsT=hTb[:], rhs=w["W2r"][:],
                             start=True, stop=True)
            nc.scalar.activation(xr2own[:, j * C2:(j + 1) * C2],
                                 xr2_ps[:, 0:C2], COPY)

        # ---------------- exchange ----------------
        nc.sync.dma_start(
            xl2loc_d[:].rearrange("(j p) f -> p j f", p=P),
            xl2own[:].rearrange("p (j f) -> p j f", f=C2))
        if debug:
            nc.sync.dma_start(
                xl2dbg_d[:].rearrange("(j p) f -> p j f", p=P),
                xl2own[:].rearrange("p (j f) -> p j f", f=C2))
        nc.gpsimd.collective_compute(
            "AllGather", mybir.AluOpType.bypass,
            replica_groups=[list(range(NCORES))],
            ins=[xl2loc_d[:]], outs=[xl2full_d[:]])
        nc.sync.dma_start(xl2pad_d[:, 0:C2], xl2full_d[:])

        # ---------------- layer 2 ----------------
        for j in range(npc):
            xl2g = gp.tile([P, cpb, H1], BF, tag="xl2g")
            gather_block(nc, xl2g, xl2pad_d, idxg, j, c1, c2, cpb, HALF, n_pad)
            eat = bp.tile([EDIM + 1, cpb * P], BF, tag="eat")
            nc.sync.dma_start(eat[:], eat_d[j, :, :])

            oh_all = bp.tile([P, cpb, P], BF, tag="oh_all")
            m2_all = bp.tile([P, cpb * C2], BF, tag="m2_all")
            m2_v = m2_all[:].rearrange("p (c f) -> p c f", f=C2)
            for c in range(cpb):
                nc.vector.tensor_scalar(
                    out=oh_all[:, c, :], in0=iota_b[:],
                    scalar1=dstf[:, j * cpb + c:j * cpb + c + 1],
                    scalar2=None, op0=ISEQ)
                ohT_ps = pb.tile([P, 1024], BF, tag="tpb")
                nc.tensor.transpose(out=ohT_ps[:, 0:P], in_=oh_all[:, c, :],
                                    identity=identb[:])
                ohT = sp.tile([P, P], BF, tag="ohT2")
                nc.scalar.activation(ohT[:], ohT_ps[:, 0:P], COPY)
                m2_ps = pm.tile([P, 512], FP, tag="m")
                nc.tensor.matmul(m2_ps[:, 0:C2], lhsT=identb[:],
                                 rhs=xl2g[:, c, 0:C2], start=True, stop=False)
                nc.tensor.matmul(m2_ps[:, 0:C2], lhsT=ohT[:],
                                 rhs=xr2own[:, j * C2:(j + 1) * C2],
                                 start=False, stop=False)
                nc.tensor.matmul(m2_ps[:, 0:C2], lhsT=eat[:, c * P:(c + 1) * P],
                                 rhs=w["We2a"][:],
                                 start=False, stop=True)
                nc.scalar.activation(m2_v[:, c, :], m2_ps[:, 0:C2], PRELU,
                                     alpha=alpha02[:])

            nc.vector.tensor_tensor(
                out=m2_v[:], in0=m2_v[:],
                in1=w["att2r"][:].unsqueeze(1).to_broadcast([P, cpb, C2]),
                op=MULT)
            lg2 = sp.tile([P, cpb], BF, tag="lg2")
            nc.vector.tensor_reduce(out=lg2[:], in_=m2_v[:],
                                    axis=mybir.AxisListType.X, op=ADD)
            x9 = bp.tile([P, cpb, C2 + 1], BF, tag="x9")
            nc.scalar.activation(x9[:, :, C2:C2 + 1], lg2[:].unsqueeze(2), EXP)
            nc.vector.tensor_tensor(
                out=x9[:, :, 0:C2], in0=xl2g[:, :, 0:C2],
                in1=x9[:, :, C2:C2 + 1].to_broadcast([P, cpb, C2]), op=MULT)

            meta2_ps = pn.tile([P, 512], FP, tag="meta")
            for c in range(cpb):
                nc.tensor.matmul(meta2_ps[:, 0:C2 + 1], lhsT=oh_all[:, c, :],
                                 rhs=x9[:, c, :],
                                 start=(c == 0), stop=(c == cpb - 1))

            # loop chunk
            ef2_ps = pf.tile([P, 512], FP, tag="pf")
            nc.tensor.matmul(ef2_ps[:, 0:C2],
                             lhsT=easumT[:, j * P:(j + 1) * P],
                             rhs=w["We2a"][:], start=True, stop=True)
            m2l = sp.tile([P, C2], BF, tag="m2l")
            nc.vector.tensor_scalar(out=m2l[:], in0=ef2_ps[:, 0:C2],
                                    scalar1=rdt[:, j:j + 1], scalar2=None,
                                    op0=MULT)
            nc.vector.tensor_tensor(out=m2l[:], in0=m2l[:],
                                    in1=xl2own[:, j * C2:(j + 1) * C2], op=ADD)
            nc.vector.tensor_tensor(out=m2l[:], in0=m2l[:],
                                    in1=xr2own[:, j * C2:(j + 1) * C2], op=ADD)
            nc.scalar.activation(m2l[:], m2l[:], PRELU, alpha=alpha02[:])
            nc.vector.tensor_tensor(out=m2l[:], in0=m2l[:], in1=w["att2r"][:],
                                    op=MULT)
            ex2l = sp.tile([P, 1], FP, tag="ex2l")
            nc.vector.tensor_reduce(out=ex2l[:], in_=m2l[:],
                                    axis=mybir.AxisListType.X, op=ADD)
            nc.scalar.activation(ex2l[:], ex2l[:], EXP)
            x9l = sp.tile([P, C2 + 1], BF, tag="x9l")
            nc.vector.tensor_scalar(out=x9l[:, 0:C2],
                                    in0=xl2own[:, j * C2:(j + 1) * C2],
                                    scalar1=ex2l[:], scalar2=None, op0=MULT)
            nc.vector.tensor_copy(x9l[:, C2:C2 + 1], ex2l[:])
            nc.tensor.matmul(meta2_ps[:, 0:C2 + 1], lhsT=identb[:], rhs=x9l[:],
                             start=False, stop=True, skip_group_check=True)

            # finalize
            rc2 = sp.tile([P, 1], FP, tag="rc2")
            nc.vector.reciprocal(rc2[:], meta2_ps[:, C2:C2 + 1])
            o2p = sp.tile([P, C2], FP, tag="o2p")
            nc.vector.tensor_scalar(out=o2p[:], in0=meta2_ps[:, 0:C2],
                                    scalar1=rc2[:], scalar2=None, op0=MULT)
            nc.vector.tensor_tensor(out=o2p[:], in0=o2p[:], in1=w["bias2row"][:],
                                    op=ADD)
            t2m = sp.tile([P, C2], FP, tag="t2m")
            nc.vector.tensor_scalar(out=t2m[:], in0=o2p[:], scalar1=0.0,
                                    scalar2=None, op0=MIN)
            u2 = sp.tile([P, C2], BF, tag="u2")
            nc.scalar.activation(u2[:], t2m[:], EXP)
            o2b = sp.tile([P, C2], BF, tag="o2b")
            nc.scalar.activation(o2b[:], o2p[:], RELU)
            nc.vector.tensor_tensor(out=o2b[:], in0=o2b[:], in1=u2[:], op=ADD)
            nc.vector.tensor_scalar(out=o2b[:], in0=o2b[:], scalar1=-1.0,
                                    scalar2=None, op0=ADD)
            o2T_ps = pb.tile([P, 1024], BF, tag="tpb")
            nc.tensor.transpose(out=o2T_ps[0:C2, 0:P], in_=o2b[:],
                                identity=identb[:])
            o2T = sp.tile([C2, P], BF, tag="o2T")
            nc.scalar.activation(o2T[:], o2T_ps[0:C2, 0:P], COPY)
            ylin_ps = pf.tile([P, 512], FP, tag="pf")
            nc.tensor.matmul(ylin_ps[:, 0:OUT], lhsT=o2T[:], rhs=w["Wlin"][:],
                             start=True, stop=False)
            nc.tensor.matmul(ylin_ps[:, 0:OUT], lhsT=ones1[:],
                             rhs=w["blin_row"][:],
                             start=False, stop=True)
            nc.scalar.activation(y_all[:, j * OUT:(j + 1) * OUT],
                                 ylin_ps[:, 0:OUT], SIGM)

        nc.sync.dma_start(
            y_d[:].rearrange("(j p) f -> p j f", p=P),
            y_all[:].rearrange("p (j f) -> p j f", f=OUT))
    return nc


# --------------------------------------------------------------------------
# runners
# --------------------------------------------------------------------------

def make_in_maps(pp, wp):
    in_maps = []
    for c in range(NCORES):
        m = dict(pp["cores"][c])
        m.pop("srcidx_emu")
        m.update(wp)
        in_maps.append(m)
    return in_maps


def run_graph(inputs, npc, backend="hw", trace=False, sim_safe=False,
              debug=False, half=32768):
    x = np.asarray(inputs["x"], np.float32)
    n = x.shape[0]
    pp = prep(x, inputs["edge_index"], inputs["edge_attr"], npc, half=half)
    wp = prep_weights(inputs)
    nc = build_nc(npc, pp["c1"], pp["c2"], pp["n_pad"], sim_safe=sim_safe,
                  debug=debug, half=half)
    nc.compile()
    in_maps = make_in_maps(pp, wp)
    info = {}
    if backend == "sim":
        from concourse.bass_interp import MultiCoreSim
        sim = MultiCoreSim(nc, num_cores=NCORES,
                           require_finite=False, require_nnan=False)
        for c in range(NCORES):
            core = sim.cores[c]
            for k, v in in_maps[c].items():
                core.tensor(k)[:] = v
        sim.simulate()
        outs = [np.asarray(sim.cores[c].tensor("y")) for c in range(NCORES)]
    else:
        from concourse.bass_utils import run_bass_kernel_spmd
        res = run_bass_kernel_spmd(nc, in_maps, list(range(NCORES)),
                                   trace=trace)
        outs = [res.results[c]["y"] for c in range(NCORES)]
        if debug:
            info["dbg"] = res.results
        info["exec_time_ns"] = res.exec_time_ns
        info["profile_json"] = getattr(res, "profile_json", None)
    yp = np.concatenate(outs, axis=0)
    y = yp[pp["permpos"][:n]]
    return np.ascontiguousarray(y), info


def kernel(**inputs):
    y, _ = run_graph(inputs, npc=49, backend="hw")
    return y


# revision 3
# speedup vs baseline: 1.0031x; 1.0031x over previous
"""GATv2 (2-layer) + linear head GNN kernel for Trainium2, 8 NeuronCores.

v2: bf16 matmuls, precomputed XL projections (stage A), one batched
indirect gather per block, node-major meta accumulation.

Per-core node space is REMAPPED: own 49 blocks first, then the other 343
blocks. srcidx1 indexes the remapped space (layer-1 gathers from the
core-private xl1_d); srcidx2 indexes the global space (layer-2 gathers
from the AllGather'd xl2full_d).
"""
import sys

sys.path.insert(0, "/opt/trn_rl_repo")

import numpy as np
import ml_dtypes
import concourse.bass as bass
import concourse.mybir as mybir
import concourse.tile as tile
from concourse import bacc
from concourse.masks import make_identity

BF16 = ml_dtypes.bfloat16

P = 128
HEADS = 4
HC = 32
H1 = HEADS * HC  # 128
C2 = 8
EDIM = 16
OUT = 8
NCORES = 8
NF = 21          # reca fields: [ex*4 | ea*16 | one]
PAD_DST = 999.0

FP = mybir.dt.float32
BF = mybir.dt.bfloat16
I32 = mybir.dt.int32


# --------------------------------------------------------------------------
# host-side preprocessing
# --------------------------------------------------------------------------

def balanced_blocks(deg, n_pad):
    import heapq

    nb = n_pad // P
    order = np.argsort(-deg, kind="stable")
    heap = [(0, b) for b in range(nb)]
    heapq.heapify(heap)
    counts = np.zeros(nb, np.int64)
    permpos = np.empty(n_pad, np.int64)
    slot_of = np.zeros(nb, np.int64)
    for node in order:
        while True:
            s, b = heapq.heappop(heap)
            if counts[b] < P:
                break
        permpos[node] = b * P + slot_of[b]
        slot_of[b] += 1
        counts[b] += 1
        if counts[b] < P:
            heapq.heappush(heap, (s + int(deg[node]), b))
    return permpos


def prep(x, edge_index, edge_attr, npc, c1c2=None, half=32768):
    n, din = x.shape
    e = edge_attr.shape[0]
    n_pad = NCORES * npc * P
    nb = n_pad // P
    HALF = min(half, n_pad)
    src = np.asarray(edge_index[0], np.int64)
    dst = np.asarray(edge_index[1], np.int64)

    deg = np.bincount(dst, minlength=n_pad).astype(np.int64)
    permpos = balanced_blocks(deg, n_pad)

    xp = np.zeros((n_pad, din), np.float32)
    xp[permpos[:n]] = np.asarray(x, np.float32)
    xpT = np.ascontiguousarray(xp.T.astype(BF16))    # [din, n_pad] global

    blk = permpos // P
    slot = permpos % P

    eb = blk[dst]
    gsrc_all = permpos[src]
    # order: by dst block, then source half (low/high), stable
    eorder = np.lexsort((gsrc_all >= HALF, eb))
    eb_s = eb[eorder]
    gsrc = gsrc_all[eorder]
    ed = dst[eorder]
    lowmask = gsrc < HALF
    n1 = np.bincount(eb_s[lowmask], minlength=nb)
    n2 = np.bincount(eb_s[~lowmask], minlength=nb)
    c1 = int(np.ceil(n1.max() / P))
    c2 = int(np.ceil(n2.max() / P))
    if c1c2 is not None:
        assert c1c2[0] >= c1 and c1c2[1] >= c2, (c1c2, c1, c2)
        c1, c2 = c1c2
    cpb = c1 + c2

    # position within block: half1 at 0.., half2 at c1*P..
    starts = np.zeros(nb + 1, np.int64)
    np.cumsum(np.bincount(eb_s, minlength=nb), out=starts[1:])
    pos_in_blk = np.arange(e) - starts[eb_s]         # order: half1 then half2
    pos = np.where(lowmask, pos_in_blk, c1 * P + pos_in_blk - n1[eb_s])
    cc = pos // P
    pp = pos % P

    g_srcl = np.zeros((nb, P, cpb), np.int64)        # half-local source idx
    g_dstf = np.full((nb, P, cpb), PAD_DST, np.float32)
    g_ea = np.zeros((nb, P, cpb, EDIM), np.float32)
    g_one = np.zeros((nb, P, cpb), np.float32)
    g_src = np.zeros((nb, P, cpb), np.int64)         # global (emulator)
    g_srcl[eb_s, pp, cc] = np.where(lowmask, gsrc, gsrc - HALF)
    g_src[eb_s, pp, cc] = gsrc
    g_dstf[eb_s, pp, cc] = slot[ed].astype(np.float32)
    g_ea[eb_s, pp, cc] = np.asarray(edge_attr, np.float32)[eorder]
    g_one[eb_s, pp, cc] = 1.0

    rd = 1.0 / np.maximum(deg.astype(np.float32), 1.0)
    rdp = np.empty(n_pad, np.float32)
    rdp[permpos] = rd
    rdt_g = rdp.reshape(nb, P).transpose(1, 0)       # [P, nb]

    def wrap16(idx_flat):
        """idx j at partition j%16, col j//16, replicated x8 -> [128, len/16]"""
        w = idx_flat.reshape(-1, 16).T               # [16, len/16]
        return np.tile(w, (8, 1))

    cores = []
    for c in range(NCORES):
        own = np.arange(c * npc, (c + 1) * npc)
        idxg = np.zeros((P, npc * cpb * 8), np.int16)
        for jj, b in enumerate(own):
            flat = g_srcl[b].T.reshape(-1)           # [cpb*P] in (cc, pp) order
            col = jj * cpb * 8
            idxg[:, col:col + c1 * 8] = wrap16(
                flat[0:c1 * P].astype(np.int16))
            idxg[:, col + c1 * 8:col + cpb * 8] = wrap16(
                flat[c1 * P:cpb * P].astype(np.int16))
        dstf = np.ascontiguousarray(
            g_dstf[own].transpose(1, 0, 2).reshape(P, npc * cpb))
        srcidx_emu = np.ascontiguousarray(
            g_src[own].transpose(1, 0, 2).reshape(P, npc * cpb))

        reca = np.zeros((npc, P, cpb, NF), np.float32)
        reca[:, :, :, 4:4 + EDIM] = g_ea[own]
        reca[:, :, :, 20] = g_one[own]
        reca = reca.reshape(npc, P, cpb * NF).astype(BF16)

        ea1 = np.concatenate([g_ea[own], g_one[own][..., None]], axis=3)
        eat = np.ascontiguousarray(
            ea1.transpose(0, 3, 2, 1)).reshape(npc, EDIM + 1, cpb * P)
        eat = eat.astype(BF16)

        rdt = np.ascontiguousarray(rdt_g[:, own])
        xpTown = np.ascontiguousarray(xpT[:, c * npc * P:(c + 1) * npc * P])
        xg4 = xp[g_src[own]].astype(BF16)            # [npc, P, cpb, 128]
        xeT = np.ascontiguousarray(
            xg4.transpose(0, 3, 2, 1)).reshape(npc, din, cpb * P)
        cores.append(dict(xpT=xpT, xpTown=xpTown, xeT=xeT, idxg=idxg,
                          dstf=dstf, reca=reca, eat=eat, rdt=rdt,
                          srcidx_emu=srcidx_emu))

    return dict(cores=cores, permpos=permpos, n_pad=n_pad, nb=nb,
                cpb=cpb, c1=c1, c2=c2)


def prep_weights(w):
    f32 = lambda a: np.asarray(a, np.float32)
    bf = lambda a: np.asarray(a, np.float32).astype(BF16)
    We1a = np.concatenate(
        [f32(w["We1"]), (f32(w["b1l"]) + f32(w["b1r"]))[None, :]], axis=0)
    We2a = np.concatenate(
        [f32(w["We2"]), (f32(w["b2l"]) + f32(w["b2r"]))[None, :]], axis=0)
    attB = np.tile(f32(w["att1"]).reshape(1, -1), (P, 1))
    att2r = np.tile(f32(w["att2"]).reshape(1, -1), (P, 1))
    E4f = np.zeros((HEADS, H1), np.float32)
    for h in range(HEADS):
        E4f[h, h * HC:(h + 1) * HC] = 1.0
    return dict(
        W1l=bf(w["W1l"]), W1r=bf(w["W1r"]), We1a=bf(We1a), attB=bf(attB),
        E4f=E4f, bias1e=(f32(w["bias1"]) + f32(w["b1l"]))[:, None],
        W2l=bf(w["W2l"]), W2r=bf(w["W2r"]), We2a=bf(We2a), att2r=bf(att2r),
        bias2row=np.tile((f32(w["bias2"]) + f32(w["b2l"]))[None, :], (P, 1)),
        Wlin=bf(w["Wlin"]), blin_row=bf(f32(w["blin"])[None, :]),
    )


# --------------------------------------------------------------------------
# numpy emulator (fp32, mirrors device dataflow; debug only)
# --------------------------------------------------------------------------

def np_forward(pp, wp, npc, lrelu_on=True):
    nb, cpb = pp["nb"], pp["cpb"]
    n_pad = pp["n_pad"]

    def lrelu(v):
        return np.where(v > 0, v, 0.2 * v) if lrelu_on else v

    def elu(v):
        return np.where(v > 0, v, np.exp(np.minimum(v, 0)) - 1.0)

    f32 = lambda a: np.asarray(a, np.float32)
    W1l, W1r = f32(wp["W1l"]), f32(wp["W1r"])
    We1a, attB = f32(wp["We1a"]), f32(wp["attB"])
    W2l, W2r, We2a = f32(wp["W2l"]), f32(wp["W2r"]), f32(wp["We2a"])
    att2r, Wlin = f32(wp["att2r"]), f32(wp["Wlin"])
    blin = f32(wp["blin_row"])[0]
    att1 = attB[0]
    att2 = att2r[0]

    xpTg = f32(pp["cores"][0]["xpT"])
    XLg = xpTg.T @ W1l                               # [n_pad, 128] global
    xl2loc = np.zeros((NCORES, npc * P, C2), np.float32)
    xr2_all = np.zeros((NCORES, npc * P, C2), np.float32)
    easumT_all = np.zeros((NCORES, EDIM + 1, npc * P), np.float32)
    for c in range(NCORES):
        cd = pp["cores"][c]
        XRo = f32(cd["xpTown"]).T @ W1r
        for j in range(npc):
            sl = slice(j * cpb, (j + 1) * cpb)
            idx = cd["srcidx_emu"][:, sl]            # [P,cpb] global
            xlg = XLg[idx]
            dstf = cd["dstf"][:, sl]
            oh = (dstf[:, :, None] == np.arange(P)[None, None, :])
            oh = oh.astype(np.float32)
            XRj = XRo[j * P:(j + 1) * P]
            eaj = f32(cd["eat"][j]).reshape(EDIM + 1, cpb, P)
            ea_e = eaj.transpose(2, 1, 0)
            m = lrelu(xlg + np.einsum("pcn,nf->pcf", oh, XRj)
                      + ea_e @ We1a)
            logits = (m * att1[None, None, :]).reshape(P, cpb, HEADS, HC).sum(-1)
            ex = np.exp(logits)
            reca = f32(cd["reca"][j]).reshape(P, cpb, NF)
            rec21 = np.concatenate([ex, reca[:, :, 4:]], axis=2)
            meta = np.einsum("pcn,pcf->nf", oh, rec21)
            easum = meta[:, 4:]
            easumT_all[c][:, j * P:(j + 1) * P] = easum.T
            rdt = cd["rdt"][:, j]
            efd = (easum @ We1a) * rdt[:, None]
            XLj = XLg[(c * npc + j) * P:(c * npc + j + 1) * P]
            mloop = lrelu(XLj + XRj + efd)
            lgl = (mloop * att1[None, :]).reshape(P, HEADS, HC).sum(-1)
            exl = np.exp(lgl)
            denom = meta[:, 0:4] + exl
            xlw = xlg * np.repeat(ex, HC, axis=2)
            numerT = np.einsum("pcf,pcn->fn", xlw, oh)
            numerT = numerT + (XLj * np.repeat(exl, HC, 1)).T
            rfull = np.repeat((1.0 / denom).T, HC, axis=0)
            hT = numerT * rfull + wp["bias1e"]
            hT = elu(hT)
            h = hT.T
            xl2loc[c, j * P:(j + 1) * P] = h @ W2l
            xr2_all[c, j * P:(j + 1) * P] = h @ W2r
    xl2full = xl2loc.reshape(-1, C2)

    y = np.zeros((n_pad, OUT), np.float32)
    for c in range(NCORES):
        cd = pp["cores"][c]
        for j in range(npc):
            sl = slice(j * cpb, (j + 1) * cpb)
            idx2 = cd["srcidx_emu"][:, sl]
            xl2g = xl2full[idx2]
            dstf = cd["dstf"][:, sl]
            oh = (dstf[:, :, None] == np.arange(P)[None, None, :])
            oh = oh.astype(np.float32)
            xr2j = xr2_all[c, j * P:(j + 1) * P]
            eaj = f32(cd["eat"][j]).reshape(EDIM + 1, cpb, P)
            ea_e = eaj.transpose(2, 1, 0)
            m2 = lrelu(xl2g + np.einsum("pcn,nf->pcf", oh, xr2j)
                       + ea_e @ We2a)
            lg2 = (m2 * att2[None, None, :]).sum(-1)
            ex2 = np.exp(lg2)
            x9 = np.concatenate([xl2g * ex2[:, :, None], ex2[:, :, None]],
                                axis=2)
            meta2 = np.einsum("pcn,pcf->nf", oh, x9)
            rdt = cd["rdt"][:, j]
            ef2 = (easumT_all[c][:, j * P:(j + 1) * P].T @ We2a) * rdt[:, None]
            xl2j = xl2loc[c, j * P:(j + 1) * P]
            m2l = lrelu(xl2j + xr2j + ef2)
            ex2l = np.exp((m2l * att2[None, :]).sum(-1))
            numer2 = meta2[:, 0:8] + xl2j * ex2l[:, None]
            denom2 = meta2[:, 8] + ex2l
            o2 = numer2 / denom2[:, None] + wp["bias2row"]
            o2 = elu(o2)
            ylin = o2 @ Wlin + blin[None, :]
            y[(c * npc + j) * P:(c * npc + j + 1) * P] = \
                1.0 / (1.0 + np.exp(-ylin))
    return y


# --------------------------------------------------------------------------
# device program
# --------------------------------------------------------------------------


GPIECE = 8  # max chunks (1024 idxs) per dma_gather call


def gather_block(nc, out_t, table_d, idxg, j, c1, c2, cpb, HALF, n_pad):
    """Gather one block's source rows in <=GPIECE-chunk dma_gather calls."""
    ioff = j * cpb * 8
    for base, nchunks, lo, hi in ((0, c1, 0, HALF), (c1, c2, HALF, n_pad)):
        for s in range(0, nchunks, GPIECE):
            k = min(GPIECE, nchunks - s)
            nc.gpsimd.dma_gather(
                out_ap=out_t[:, base + s:base + s + k, :],
                in_ap=table_d[lo:hi, :],
                idxs_ap=idxg[:, ioff + (base + s) * 8:
                             ioff + (base + s + k) * 8],
                num_idxs=k * P, num_idxs_reg=k * P, elem_size=H1)


def build_nc(npc, c1, c2, n_pad, sim_safe=False, debug=False,
             half=32768):
    from concourse import library_config
    nc = bacc.Bacc("TRN2", target_bir_lowering=False)
    cpb = c1 + c2
    npcP = npc * P
    nb = n_pad // P
    HALF = min(half, n_pad)
    GB = 7                      # blocks per stage-A group
    assert npc % GB == 0 and nb % GB == 0

    xpTown_d = nc.dram_tensor("xpTown", [H1, npcP], BF, kind="ExternalInput")
    xeT_d = nc.dram_tensor("xeT", [npc, H1, cpb * P], BF, kind="ExternalInput")
    idxg_d = nc.dram_tensor("idxg", [P, npc * cpb * 8], mybir.dt.int16,
                            kind="ExternalInput")
    dstf_d = nc.dram_tensor("dstf", [P, npc * cpb], FP, kind="ExternalInput")
    reca_d = nc.dram_tensor("reca", [npc, P, cpb * NF], BF, kind="ExternalInput")
    eat_d = nc.dram_tensor("eat", [npc, EDIM + 1, cpb * P], BF,
                           kind="ExternalInput")
    rdt_d = nc.dram_tensor("rdt", [P, npc], FP, kind="ExternalInput")
    wnames = dict(
        W1l=([H1, H1], BF), W1r=([H1, H1], BF), We1a=([EDIM + 1, H1], BF),
        attB=([P, H1], BF), E4f=([HEADS, H1], FP), bias1e=([H1, 1], FP),
        W2l=([H1, C2], BF), W2r=([H1, C2], BF), We2a=([EDIM + 1, C2], BF),
        att2r=([P, C2], BF), bias2row=([P, C2], FP), Wlin=([C2, C2], BF),
        blin_row=([1, C2], BF),
    )
    wd = {k: nc.dram_tensor(k, sh, dt, kind="ExternalInput")
          for k, (sh, dt) in wnames.items()}
    y_d = nc.dram_tensor("y", [npcP, OUT], FP, kind="ExternalOutput")
    xl2loc_d = nc.dram_tensor("xl2loc", [npcP, C2], BF)
    if debug:
        xl2dbg_d = nc.dram_tensor("xl2dbg", [npcP, C2], BF,
                                  kind="ExternalOutput")
        xlgdbg_d = nc.dram_tensor("xlgdbg", [P, cpb * H1], BF,
                                  kind="ExternalOutput")
        mdbg_d = nc.dram_tensor("mdbg", [P, cpb * H1], BF,
                                kind="ExternalOutput")
        ohdbg_d = nc.dram_tensor("ohdbg", [P, cpb * P], BF,
                                 kind="ExternalOutput")
        recdbg_d = nc.dram_tensor("recdbg", [P, cpb * NF], BF,
                                  kind="ExternalOutput")
    xl2full_d = nc.dram_tensor("xl2full", [n_pad, C2], BF, addr_space="Shared")
    xl2pad_d = nc.dram_tensor("xl2pad", [n_pad, H1], BF)

    PRELU = mybir.ActivationFunctionType.Prelu
    if sim_safe:
        PRELU = mybir.ActivationFunctionType.Copy
    EXP = mybir.ActivationFunctionType.Exp
    RELU = mybir.ActivationFunctionType.Relu
    COPY = mybir.ActivationFunctionType.Copy
    SIGM = mybir.ActivationFunctionType.Sigmoid
    ADD = mybir.AluOpType.add
    MULT = mybir.AluOpType.mult
    MIN = mybir.AluOpType.min
    ISEQ = mybir.AluOpType.is_equal

    from contextlib import ExitStack

    with tile.TileContext(nc) as tc, ExitStack() as stack, \
            nc.allow_low_precision(reason="bf16 logits reduce"):
        cp = stack.enter_context(tc.tile_pool(name="consts", bufs=1))
        bp = stack.enter_context(tc.tile_pool(name="big", bufs=2))
        gp = stack.enter_context(tc.tile_pool(name="gath", bufs=3))
        sp = stack.enter_context(tc.tile_pool(name="small", bufs=3))
        pf = stack.enter_context(tc.tile_pool(name="pf", bufs=2, space="PSUM"))
        pb = stack.enter_context(tc.tile_pool(name="pb", bufs=2, space="PSUM"))
        pn = stack.enter_context(tc.tile_pool(name="pn", bufs=1, space="PSUM"))
        pm = stack.enter_context(tc.tile_pool(name="pm", bufs=2, space="PSUM"))

        identb = cp.tile([P, P], BF)
        make_identity(nc, identb[:])
        identf = cp.tile([P, P], FP)
        make_identity(nc, identf[:])
        iota_i = cp.tile([P, P], I32)
        nc.gpsimd.iota(iota_i[:], pattern=[[1, P]], base=0, channel_multiplier=0)
        iota_b = cp.tile([P, P], BF)
        nc.vector.tensor_copy(iota_b[:], iota_i[:])
        alpha02 = cp.tile([P, 1], FP)
        nc.vector.memset(alpha02[:], 0.2 if not sim_safe else 1.0)
        ones1 = cp.tile([1, P], BF)
        nc.vector.memset(ones1[:], 1.0)
        w = {}
        for k, (sh, dt) in wnames.items():
            w[k] = cp.tile(sh, dt, name=f"w_{k}", tag=f"w_{k}")
            nc.sync.dma_start(w[k][:], wd[k][:])
        rdt = cp.tile([P, npc], FP)
        nc.sync.dma_start(rdt[:], rdt_d[:])
        idxg = cp.tile([P, npc * cpb * 8], mybir.dt.int16)
        nc.sync.dma_start(idxg[:], idxg_d[:])
        dstf = cp.tile([P, npc * cpb], FP)
        nc.sync.dma_start(dstf[:], dstf_d[:])
        nc.gpsimd.load_library(library_config.mlp)

        XLown = cp.tile([P, npc * P], BF)
        XRown = cp.tile([P, npc * P], BF)
        easumT = cp.tile([EDIM + 1, npc * P], BF)
        xl2own = cp.tile([P, npc * C2], BF)
        xr2own = cp.tile([P, npc * C2], BF)
        y_all = cp.tile([P, npc * OUT], FP)

        # ---------------- stage A: XL = x @ W1l for all blocks ------------
        ngrp_own = npc // GB
        for g in range(ngrp_own):
            xt = bp.tile([P, GB * P], BF, tag="xpT_in")
            nc.sync.dma_start(xt[:], xpTown_d[:, g * GB * P:(g + 1) * GB * P])
            for k in range(GB):
                b0 = g * GB + k
                xl_ps = pf.tile([P, 512], FP, tag="pf")
                nc.tensor.matmul(xl_ps[:, 0:H1], lhsT=xt[:, k * P:(k + 1) * P],
                                 rhs=w["W1l"][:], start=True, stop=True)
                nc.scalar.activation(XLown[:, b0 * P:(b0 + 1) * P],
                                     xl_ps[:, 0:H1], COPY)
                xr_ps = pf.tile([P, 512], FP, tag="pf")
                nc.tensor.matmul(xr_ps[:, 0:H1], lhsT=xt[:, k * P:(k + 1) * P],
                                 rhs=w["W1r"][:], start=True, stop=True)
                nc.scalar.activation(XRown[:, b0 * P:(b0 + 1) * P],
                                     xr_ps[:, 0:H1], COPY)

        # ---------------- layer 1 ----------------
        for j in range(npc):
            xlg = gp.tile([P, cpb, H1], BF, tag="xlg")
            xeT = bp.tile([P, cpb * P], BF, tag="xeT")
            nc.sync.dma_start(xeT[:], xeT_d[j, :, :])
            reca = bp.tile([P, cpb * NF], BF, tag="reca")
            nc.sync.dma_start(reca[:], reca_d[j, :, :])
            rec_v = reca[:].rearrange("p (c f) -> p c f", f=NF)
            eat = bp.tile([EDIM + 1, cpb * P], BF, tag="eat")
            nc.sync.dma_start(eat[:], eat_d[j, :, :])

            oh_all = bp.tile([P, cpb, P], BF, tag="oh_all")
            ohT_all = bp.tile([P, cpb, P], BF, tag="ohT_all")
            m_all = bp.tile([P, cpb * H1], BF, tag="m_all")
            m_v = m_all[:].rearrange("p (c f) -> p c f", f=H1)
            xlg_v = xlg[:]

            for c in range(cpb):
                nc.vector.tensor_scalar(
                    out=oh_all[:, c, :], in0=iota_b[:],
                    scalar1=dstf[:, j * cpb + c:j * cpb + c + 1],
                    scalar2=None, op0=ISEQ)
                ohT_ps = pb.tile([P, 1024], BF, tag="tpb")
                nc.tensor.transpose(out=ohT_ps[:, 0:P], in_=oh_all[:, c, :],
                                    identity=identb[:])
                nc.scalar.activation(ohT_all[:, c, :], ohT_ps[:, 0:P], COPY)
                xl_ps = pf.tile([P, 512], FP, tag="pf")
                nc.tensor.matmul(xl_ps[:, 0:H1],
                                 lhsT=xeT[:, c * P:(c + 1) * P],
                                 rhs=w["W1l"][:], start=True, stop=True)
                nc.scalar.activation(xlg_v[:, c, :], xl_ps[:, 0:H1], COPY)
                m_ps = pm.tile([P, 512], FP, tag="m")
                nc.tensor.matmul(m_ps[:, 0:H1], lhsT=identb[:],
                                 rhs=xlg_v[:, c, :], start=True, stop=False)
                nc.tensor.matmul(m_ps[:, 0:H1], lhsT=ohT_all[:, c, :],
                                 rhs=XRown[:, j * P:(j + 1) * P],
                                 start=False, stop=False)
                nc.tensor.matmul(m_ps[:, 0:H1], lhsT=eat[:, c * P:(c + 1) * P],
                                 rhs=w["We1a"][:],
                                 start=False, stop=True)
                nc.scalar.activation(m_v[:, c, :], m_ps[:, 0:H1], PRELU,
                                     alpha=alpha02[:])

            # logits -> ex (into reca cols 0:4)
            nc.vector.tensor_tensor(
                out=m_v[:], in0=m_v[:],
                in1=w["attB"][:].unsqueeze(1).to_broadcast([P, cpb, H1]),
                op=MULT)
            logits = sp.tile([P, cpb * HEADS], BF, tag="logits")
            nc.vector.tensor_reduce(
                out=logits[:].rearrange("p (c h) -> p c h", h=HEADS),
                in_=m_all[:].rearrange("p (c h k) -> p c h k", h=HEADS, k=HC),
                axis=mybir.AxisListType.X, op=ADD)
            nc.scalar.activation(
                rec_v[:, :, 0:HEADS],
                logits[:].rearrange("p (c h) -> p c h", h=HEADS), EXP)
            # xlw = xlg * ex (in place)
            nc.vector.tensor_tensor(
                out=xlg_v.rearrange("p c (h k) -> p c h k", h=HEADS),
                in0=xlg_v.rearrange("p c (h k) -> p c h k", h=HEADS),
                in1=rec_v[:, :, 0:HEADS].unsqueeze(3)
                    .to_broadcast([P, cpb, HEADS, HC]),
                op=MULT)

            if debug and j == 0:
                nc.sync.dma_start(xlgdbg_d[:],
                                  xlg[:].rearrange("p c f -> p (c f)"))

                nc.sync.dma_start(mdbg_d[:], m_all[:])
                nc.sync.dma_start(ohdbg_d[:],
                                  oh_all[:].rearrange("p c f -> p (c f)"))
                nc.sync.dma_start(recdbg_d[:], reca[:])
            numerT_ps = pn.tile([P, 512], FP, tag="numerT")
            meta_ps = pn.tile([P, 512], FP, tag="meta")
            for c in range(cpb):
                nc.tensor.matmul(numerT_ps[:, 0:P], lhsT=xlg_v[:, c, :],
                                 rhs=oh_all[:, c, :],
                                 start=(c == 0), stop=(c == cpb - 1))
                nc.tensor.matmul(meta_ps[:, 0:NF], lhsT=oh_all[:, c, :],
                                 rhs=rec_v[:, c, :],
                                 start=(c == 0), stop=(c == cpb - 1))

            # loop chunk (self loop), node-major
            easum_nm = sp.tile([P, EDIM + 1], BF, tag="easum_nm")
            nc.scalar.activation(easum_nm[:], meta_ps[:, 4:NF], COPY)
            eaT_ps = pb.tile([P, 1024], BF, tag="tpb")
            nc.tensor.transpose(out=eaT_ps[0:EDIM + 1, 0:P], in_=easum_nm[:],
                                identity=identb[:])
            nc.scalar.activation(easumT[:, j * P:(j + 1) * P],
                                 eaT_ps[0:EDIM + 1, 0:P], COPY)
            efd_ps = pf.tile([P, 512], FP, tag="pf")
            nc.tensor.matmul(efd_ps[:, 0:H1],
                             lhsT=easumT[:, j * P:(j + 1) * P],
                             rhs=w["We1a"][:], start=True, stop=True)
            efd = sp.tile([P, H1], BF, tag="efd")
            nc.vector.tensor_scalar(out=efd[:], in0=efd_ps[:, 0:H1],
                                    scalar1=rdt[:, j:j + 1], scalar2=None,
                                    op0=MULT)
            ml_ps = pf.tile([P, 512], FP, tag="pf")
            nc.tensor.matmul(ml_ps[:, 0:H1], lhsT=identb[:],
                             rhs=XLown[:, j * P:(j + 1) * P],
                             start=True, stop=False)
            nc.tensor.matmul(ml_ps[:, 0:H1], lhsT=identb[:],
                             rhs=XRown[:, j * P:(j + 1) * P],
                             start=False, stop=False)
            nc.tensor.matmul(ml_ps[:, 0:H1], lhsT=identb[:], rhs=efd[:],
                             start=False, stop=True)
            ml = sp.tile([P, H1], BF, tag="ml")
            nc.scalar.activation(ml[:], ml_ps[:, 0:H1], PRELU, alpha=alpha02[:])
            nc.vector.tensor_tensor(out=ml[:], in0=ml[:], in1=w["attB"][:],
                                    op=MULT)
            lgl = sp.tile([P, HEADS], BF, tag="lgl")
            nc.vector.tensor_reduce(
                out=lgl[:], in_=ml[:].rearrange("p (h k) -> p h k", h=HEADS),
                axis=mybir.AxisListType.X, op=ADD)
            exl = sp.tile([P, HEADS], BF, tag="exl")
            nc.scalar.activation(exl[:], lgl[:], EXP)
            nc.tensor.matmul(meta_ps[:, 0:HEADS], lhsT=identb[:], rhs=exl[:],
                             start=False, stop=True, skip_group_check=True)
            xlwl = sp.tile([P, H1], BF, tag="xlwl")
            nc.vector.tensor_tensor(
                out=xlwl[:].rearrange("p (h k) -> p h k", h=HEADS),
                in0=XLown[:, j * P:(j + 1) * P]
                    .rearrange("p (h k) -> p h k", h=HEADS),
                in1=exl[:].unsqueeze(2).to_broadcast([P, HEADS, HC]),
                op=MULT)
            nc.tensor.matmul(numerT_ps[:, 0:P], lhsT=xlwl[:], rhs=identb[:],
                             start=False, stop=True, skip_group_check=True)

            # finalize block: hT = elu(numerT * rfull + bias1e)
            recip = sp.tile([P, HEADS], FP, tag="recip")
            nc.vector.reciprocal(recip[:], meta_ps[:, 0:HEADS])
            rcT_ps = pf.tile([P, 512], FP, tag="pf")
            nc.tensor.transpose(out=rcT_ps[0:HEADS, 0:P], in_=recip[:],
                                identity=identf[:])
            rcT = sp.tile([HEADS, P], FP, tag="rcT")
            nc.scalar.activation(rcT[:], rcT_ps[0:HEADS, 0:P], COPY)
            rfull_ps = pf.tile([P, 512], FP, tag="pf")
            nc.tensor.matmul(rfull_ps[:, 0:P], lhsT=w["E4f"][:], rhs=rcT[:],
                             start=True, stop=True)
            rfull = sp.tile([P, P], FP, tag="rfull")
            nc.scalar.activation(rfull[:], rfull_ps[:, 0:P], COPY)
            hT = sp.tile([P, P], FP, tag="hT")
            nc.vector.tensor_tensor(out=hT[:], in0=numerT_ps[:, 0:P],
                                    in1=rfull[:], op=MULT)
            tmin = sp.tile([P, P], FP, tag="tmin")
            nc.vector.tensor_scalar(out=tmin[:], in0=hT[:],
                                    scalar1=w["bias1e"][:], scalar2=0.0,
                                    op0=ADD, op1=MIN)
            ue = sp.tile([P, P], BF, tag="ue")
            nc.scalar.activation(ue[:], tmin[:], EXP)
            hTb = sp.tile([P, P], BF, tag="hTb")
            nc.scalar.activation(hTb[:], hT[:], RELU, bias=w["bias1e"][:])
            nc.vector.tensor_tensor(out=hTb[:], in0=hTb[:], in1=ue[:], op=ADD)
            nc.vector.tensor_scalar(out=hTb[:], in0=hTb[:], scalar1=-1.0,
                                    scalar2=None, op0=ADD)

            xl2_ps = pf.tile([P, 512], FP, tag="pf")
            nc.tensor.matmul(xl2_ps[:, 0:C2], lhsT=hTb[:], rhs=w["W2l"][:],
                             start=True, stop=True)
            nc.scalar.activation(xl2own[:, j * C2:(j + 1) * C2],
                                 xl2_ps[:, 0:C2], COPY)
            xr2_ps = pf.tile([P, 512], FP, tag="pf")
            nc.tensor.matmul(xr2_ps[:, 0:C2], lhsT=hTb[:], rhs=w["W2r"][:],
                             start=True, stop=True)
            nc.scalar.activation(xr2own[:, j * C2:(j + 1) * C2],
                                 xr2_ps[:, 0:C2], COPY)

        # ---------------- exchange ----------------
        nc.sync.dma_start(
            xl2loc_d[:].rearrange("(j p) f -> p j f", p=P),
            xl2own[:].rearrange("p (j f) -> p j f", f=C2))
        if debug:
            nc.sync.dma_start(
                xl2dbg_d[:].rearrange("(j p) f -> p j f", p=P),
                xl2own[:].rearrange("p (j f) -> p j f", f=C2))
        nc.gpsimd.collective_compute(
            "AllGather", mybir.AluOpType.bypass,
            replica_groups=[list(range(NCORES))],
            ins=[xl2loc_d[:]], outs=[xl2full_d[:]])
        nc.sync.dma_start(xl2pad_d[:, 0:C2], xl2full_d[:])

        # ---------------- layer 2 ----------------
        for j in range(npc):
            xl2g = gp.tile([P, cpb, H1], BF, tag="xl2g")
            gather_block(nc, xl2g, xl2pad_d, idxg, j, c1, c2, cpb, HALF, n_pad)
            eat = bp.tile([EDIM + 1, cpb * P], BF, tag="eat")
            nc.sync.dma_start(eat[:], eat_d[j, :, :])

            oh_all = bp.tile([P, cpb, P], BF, tag="oh_all")
            m2_all = bp.tile([P, cpb * C2], BF, tag="m2_all")
            m2_v = m2_all[:].rearrange("p (c f) -> p c f", f=C2)
            for c in range(cpb):
                nc.vector.tensor_scalar(
                    out=oh_all[:, c, :], in0=iota_b[:],
                    scalar1=dstf[:, j * cpb + c:j * cpb + c + 1],
                    scalar2=None, op0=ISEQ)
                ohT_ps = pb.tile([P, 1024], BF, tag="tpb")
                nc.tensor.transpose(out=ohT_ps[:, 0:P], in_=oh_all[:, c, :],
                                    identity=identb[:])
                ohT = sp.tile([P, P], BF, tag="ohT2")
                nc.scalar.activation(ohT[:], ohT_ps[:, 0:P], COPY)
                m2_ps = pm.tile([P, 512], FP, tag="m")
                nc.tensor.matmul(m2_ps[:, 0:C2], lhsT=identb[:],
                                 rhs=xl2g[:, c, 0:C2], start=True, stop=False)
                nc.tensor.matmul(m2_ps[:, 0:C2], lhsT=ohT[:],
                                 rhs=xr2own[:, j * C2:(j + 1) * C2],
                                 start=False, stop=False)
                nc.tensor.matmul(m2_ps[:, 0:C2], lhsT=eat[:, c * P:(c + 1) * P],
                                 rhs=w["We2a"][:],
                                 start=False, stop=True)
                nc.scalar.activation(m2_v[:, c, :], m2_ps[:, 0:C2], PRELU,
                                     alpha=alpha02[:])

            nc.vector.tensor_tensor(
                out=m2_v[:], in0=m2_v[:],
                in1=w["att2r"][:].unsqueeze(1).to_broadcast([P, cpb, C2]),
                op=MULT)
            lg2 = sp.tile([P, cpb], BF, tag="lg2")
            nc.vector.tensor_reduce(out=lg2[:], in_=m2_v[:],
                                    axis=mybir.AxisListType.X, op=ADD)
            x9 = bp.tile([P, cpb, C2 + 1], BF, tag="x9")
            nc.scalar.activation(x9[:, :, C2:C2 + 1], lg2[:].unsqueeze(2), EXP)
            nc.vector.tensor_tensor(
                out=x9[:, :, 0:C2], in0=xl2g[:, :, 0:C2],
                in1=x9[:, :, C2:C2 + 1].to_broadcast([P, cpb, C2]), op=MULT)

            meta2_ps = pn.tile([P, 512], FP, tag="meta")
            for c in range(cpb):
                nc.tensor.matmul(meta2_ps[:, 0:C2 + 1], lhsT=oh_all[:, c, :],
                                 rhs=x9[:, c, :],
                                 start=(c == 0), stop=(c == cpb - 1))

            # loop chunk
            ef2_ps = pf.tile([P, 512], FP, tag="pf")
            nc.tensor.matmul(ef2_ps[:, 0:C2],
                             lhsT=easumT[:, j * P:(j + 1) * P],
                             rhs=w["We2a"][:], start=True, stop=True)
            m2l = sp.tile([P, C2], BF, tag="m2l")
            nc.vector.tensor_scalar(out=m2l[:], in0=ef2_ps[:, 0:C2],
                                    scalar1=rdt[:, j:j + 1], scalar2=None,
                                    op0=MULT)
            nc.vector.tensor_tensor(out=m2l[:], in0=m2l[:],
                                    in1=xl2own[:, j * C2:(j + 1) * C2], op=ADD)
            nc.vector.tensor_tensor(out=m2l[:], in0=m2l[:],
                                    in1=xr2own[:, j * C2:(j + 1) * C2], op=ADD)
            nc.scalar.activation(m2l[:], m2l[:], PRELU, alpha=alpha02[:])
            nc.vector.tensor_tensor(out=m2l[:], in0=m2l[:], in1=w["att2r"][:],
                                    op=MULT)
            ex2l = sp.tile([P, 1], FP, tag="ex2l")
            nc.vector.tensor_reduce(out=ex2l[:], in_=m2l[:],
                                    axis=mybir.AxisListType.X, op=ADD)
            nc.scalar.activation(ex2l[:], ex2l[:], EXP)
            x9l = sp.tile([P, C2 + 1], BF, tag="x9l")
            nc.vector.tensor_scalar(out=x9l[:, 0:C2],
                                    in0=xl2own[:, j * C2:(j + 1) * C2],
                                    scalar1=ex2l[:], scalar2=None, op0=MULT)
            nc.vector.tensor_copy(x9l[:, C2:C2 + 1], ex2l[:])
            nc.tensor.matmul(meta2_ps[:, 0:C2 + 1], lhsT=identb[:], rhs=x9l[:],
                             start=False, stop=True, skip_group_check=True)

            # finalize
            rc2 = sp.tile([P, 1], FP, tag="rc2")
            nc.vector.reciprocal(rc2[:], meta2_ps[:, C2:C2 + 1])
            o2p = sp.tile([P, C2], FP, tag="o2p")
            nc.vector.tensor_scalar(out=o2p[:], in0=meta2_ps[:, 0:C2],
                                    scalar1=rc2[:], scalar2=None, op0=MULT)
            nc.vector.tensor_tensor(out=o2p[:], in0=o2p[:], in1=w["bias2row"][:],
                                    op=ADD)
            t2m = sp.tile([P, C2], FP, tag="t2m")
            nc.vector.tensor_scalar(out=t2m[:], in0=o2p[:], scalar1=0.0,
                                    scalar2=None, op0=MIN)
            u2 = sp.tile([P, C2], BF, tag="u2")
            nc.scalar.activation(u2[:], t2m[:], EXP)
            o2b = sp.tile([P, C2], BF, tag="o2b")
            nc.scalar.activation(o2b[:], o2p[:], RELU)
            nc.vector.tensor_tensor(out=o2b[:], in0=o2b[:], in1=u2[:], op=ADD)
            nc.vector.tensor_scalar(out=o2b[:], in0=o2b[:], scalar1=-1.0,
                                    scalar2=None, op0=ADD)
            o2T_ps = pb.tile([P, 1024], BF, tag="tpb")
            nc.tensor.transpose(out=o2T_ps[0:C2, 0:P], in_=o2b[:],
                                identity=identb[:])
            o2T = sp.tile([C2, P], BF, tag="o2T")
            nc.scalar.activation(o2T[:], o2T_ps[0:C2, 0:P], COPY)
            ylin_ps = pf.tile([P, 512], FP, tag="pf")
            nc.tensor.matmul(ylin_ps[:, 0:OUT], lhsT=o2T[:], rhs=w["Wlin"][:],
                             start=True, stop=False)
            nc.tensor.matmul(ylin_ps[:, 0:OUT], lhsT=ones1[:],
                             rhs=w["blin_row"][:],
                             start=False, stop=True)
            nc.scalar.activation(y_all[:, j * OUT:(j + 1) * OUT],
                                 ylin_ps[:, 0:OUT], SIGM)

        nc.sync.dma_start(
            y_d[:].rearrange("(j p) f -> p j f", p=P),
            y_all[:].rearrange("p (j f) -> p j f", f=OUT))
    return nc


# --------------------------------------------------------------------------
# runners
# --------------------------------------------------------------------------

def make_in_maps(pp, wp):
    in_maps = []
    for c in range(NCORES):
        m = dict(pp["cores"][c])
        m.pop("srcidx_emu")
        m.pop("xpT")
        m.update(wp)
        in_maps.append(m)
    return in_maps


def run_graph(inputs, npc, backend="hw", trace=False, sim_safe=False,
              debug=False, half=32768):
    x = np.asarray(inputs["x"], np.float32)
    n = x.shape[0]
    pp = prep(x, inputs["edge_index"], inputs["edge_attr"], npc, half=half)
    wp = prep_weights(inputs)
    nc = build_nc(npc, pp["c1"], pp["c2"], pp["n_pad"], sim_safe=sim_safe,
                  debug=debug, half=half)
    nc.compile()
    in_maps = make_in_maps(pp, wp)
    info = {}
    if backend == "sim":
        from concourse.bass_interp import MultiCoreSim
        sim = MultiCoreSim(nc, num_cores=NCORES,
                           require_finite=False, require_nnan=False)
        for c in range(NCORES):
            core = sim.cores[c]
            for k, v in in_maps[c].items():
                core.tensor(k)[:] = v
        sim.simulate()
        outs = [np.asarray(sim.cores[c].tensor("y")) for c in range(NCORES)]
    else:
        from concourse.bass_utils import run_bass_kernel_spmd
        res = run_bass_kernel_spmd(nc, in_maps, list(range(NCORES)),
                                   trace=trace)
        outs = [res.results[c]["y"] for c in range(NCORES)]
        if debug:
            info["dbg"] = res.results
        info["exec_time_ns"] = res.exec_time_ns
        info["profile_json"] = getattr(res, "profile_json", None)
    yp = np.concatenate(outs, axis=0)
    y = yp[pp["permpos"][:n]]
    return np.ascontiguousarray(y), info


def kernel(**inputs):
    y, _ = run_graph(inputs, npc=49, backend="hw")
    return y
